# revision 19
# baseline (speedup 1.0000x reference)
"""Trainium2 Bass kernel for nn_BlockDiagonalLinear_text (hyperbolic block-diag linear).

Math: every per-row operation in the reference is a scalar row-scaling of
  y = x @ blockdiag(W_1..W_16).T
and the scalar chain collapses via artanh(tanh(t)) = t:
  out = 10 * clamp(y_n * k1, 1e-6, CB) / y_n * y     (y_n = ||y||)
  k1  = min(0.1*uc, CA) / uc,  uc = max(||x||, 1e-5)
  CA  = artanh(f32(1 - 1e-5))   (expmap tanh always lands in the artanh clip)
  CB  = artanh(f32(0.1) * f32(9.99))  (the _project maxnorm cancels pf*ttx)
k1 depends only on x, so it is precomputed on the host (one fused scalar
per row) alongside the input layout transform.

Device layout (per core, data-parallel over rows: 8192 rows -> 8 x 1024):
  xt  [128, 8*32*128] bf16 -- x pre-transposed on host so each matmul's
      stationary operand xt[:, i*4096 + kc*128 :+128] = x[tile rows, k-chunk].T
      DMAs at full rate (8 KiB/partition/tile contiguous), no PE transposes,
      no PSUM->SBUF cast copies.
  w   [128, 8192] bf16 -- w[p, kc*256+j] = W[kc//2, j, (kc%2)*128+p]
  k1  [128, 8] f32 per-tile row scalars
Per 128-row tile: 32 bf16 matmuls (2 per 256-col block) into 8 PSUM banks
[128,512]; ACT Square+accum per bank -> qy partials; DVE copies bank->SBUF;
tiny DVE chain -> alm; DVE scaled in-place; DMA out f32.
Single ACT table (Square/Rsqrt/Copy) -> zero ACT_TABLE_LOADs.
"""
import sys
import numpy as np

for _p in ("/opt/trn_rl_repo", "/root/.axon_site/_ro/trn_rl_repo"):
    if _p not in sys.path:
        sys.path.append(_p)

import ml_dtypes
import concourse.bass as bass
import concourse.bacc as bacc
import concourse.mybir as mybir
from concourse import tile
from concourse.bass_utils import run_bass_kernel_spmd

R, BS = 16, 256           # 16 diagonal blocks of 256x256
D = R * BS                # 4096
P = 128                   # partitions
N_CORES = 8
ROWS_TOTAL = 4 * 2048     # 8192
ROWS_CORE = ROWS_TOTAL // N_CORES   # 1024
NT = ROWS_CORE // P       # 8 tiles of 128 rows per core
NKC = D // P              # 32 k-chunks of 128
NB = 8                    # PSUM banks per tile (512 cols each)
BANK = 512

f32 = mybir.dt.float32
bf16 = mybir.dt.bfloat16
AF = mybir.ActivationFunctionType
OP = mybir.AluOpType
AX = None  # set lazily (bass_rust import)

CA = 6.10235526389634     # artanh(f32(1 - 1e-5))
CB = 3.800207607813536    # artanh(f32(0.1) * f32((1-1e-3)/0.1))


def build_nc():
    import bass_rust
    nc = bacc.Bacc()
    xt_d = nc.declare_dram_parameter("xt", [P, NT * D], bf16, isOutput=False)
    w_d = nc.declare_dram_parameter("w", [P, 2 * R * BS], bf16, isOutput=False)
    k1_d = nc.declare_dram_parameter("k1", [P, NT], f32, isOutput=False)
    out_d = nc.declare_dram_parameter("out", [ROWS_CORE, D], f32, isOutput=True)

    with tile.TileContext(nc) as tc:
        with (
            tc.tile_pool(name="xtpool", bufs=1) as xtpool,
            tc.tile_pool(name="wpool", bufs=1) as wpool,
            tc.tile_pool(name="kpool", bufs=1) as kpool,
            tc.tile_pool(name="ypool", bufs=4) as ypool,
            tc.tile_pool(name="scrpool", bufs=1) as scrpool,
            tc.tile_pool(name="stats", bufs=4) as stats,
            tc.tile_pool(name="pst", bufs=4, space="PSUM") as pst,
        ):
            # input DMAs, ordered so tile-0 bank-0 can start ASAP
            wts = [wpool.tile([P, 8 * BS], bf16, tag=f"w{g}", name=f"w_{g}")
                   for g in range(4)]
            xts = [xtpool.tile([P, D], bf16, tag=f"xt{i}", name=f"xt_{i}")
                   for i in range(NT)]
            k1_sb = kpool.tile([P, NT], f32, name="k1_sb")

            nc.sync.dma_start(out=wts[0][:], in_=w_d[:, 0:8 * BS])
            for g in range(4):  # xt0 in quarters for early dependency release
                nc.sync.dma_start(out=xts[0][:, g * 1024:(g + 1) * 1024],
                                  in_=xt_d[:, g * 1024:(g + 1) * 1024])
            for g in range(1, 4):
                nc.sync.dma_start(out=wts[g][:],
                                  in_=w_d[:, g * 8 * BS:(g + 1) * 8 * BS])
            nc.sync.dma_start(out=k1_sb[:], in_=k1_d[:])
            for i in range(1, NT):
                nc.sync.dma_start(out=xts[i][:], in_=xt_d[:, i * D:(i + 1) * D])

            scr = scrpool.tile([P, D], f32, name="scr")

            def st(shape, tag):
                return stats.tile(shape, f32, tag=tag, name=tag)

            V = nc.vector
            for i in range(NT):
                y_sb = ypool.tile([P, D], f32, tag="y", name=f"y_{i}")
                for bb in range(NB // 2):
                    # 2-bank PSUM tile: 4 blocks, 8 matmuls
                    py = pst.tile([P, 2 * BANK], f32, tag="py", name=f"py_{i}_{bb}")
                    for blk in range(4):
                        r = 4 * bb + blk
                        for c in range(2):
                            kc = 2 * r + c
                            nc.tensor.matmul(
                                py[:, blk * BS:(blk + 1) * BS],
                                xts[i][:, kc * P:(kc + 1) * P],
                                wts[kc // 8][:, (kc % 8) * BS:(kc % 8 + 1) * BS],
                                start=(c == 0), stop=(c == 1),
                            )
                    # y copy (2 banks at once): ACT reads PSUM fastest
                    nc.scalar.activation(y_sb[:, bb * 2 * BANK:(bb + 1) * 2 * BANK],
                                         py[:], AF.Copy)
                    # qy half-partials on DVE (SBUF 2x mode) as halves land
                    if bb == 1:
                        qa = st([P, 1], "qa")
                        V.scalar_tensor_tensor(
                            out=scr[:, 0:D // 2], in0=y_sb[:, 0:D // 2],
                            scalar=1.0, in1=y_sb[:, 0:D // 2],
                            op0=OP.mult, op1=OP.mult, accum_out=qa[:])
                    elif bb == 3:
                        qb = st([P, 1], "qb")
                        V.scalar_tensor_tensor(
                            out=scr[:, D // 2:D], in0=y_sb[:, D // 2:D],
                            scalar=1.0, in1=y_sb[:, D // 2:D],
                            op0=OP.mult, op1=OP.mult, accum_out=qb[:])

                # ---- collapsed per-row chain ----
                qy = st([P, 1], "qy")
                V.tensor_tensor(qy[:], qa[:], qb[:], OP.add)
                qyc = st([P, 1], "qyc")
                V.tensor_scalar_max(qyc[:], qy[:], 1e-38)
                y_n = st([P, 1], "y_n")
                nc.scalar.activation(y_n[:], qyc[:], AF.Sqrt)
                ry = st([P, 1], "ry")
                V.reciprocal(ry[:], y_n[:])
                w2 = st([P, 1], "w2")
                V.tensor_tensor(w2[:], y_n[:], k1_sb[:, i:i + 1], OP.mult)
                g_ = st([P, 1], "g_")
                V.tensor_scalar(out=g_[:], in0=w2[:], scalar1=1e-6, scalar2=CB,
                                op0=OP.max, op1=OP.min)
                a_ = st([P, 1], "a_")
                V.tensor_tensor(a_[:], g_[:], ry[:], OP.mult)
                alm = st([P, 1], "alm")
                V.scalar_tensor_tensor(out=alm[:], in0=qy[:], scalar=0.0,
                                       in1=a_[:], op0=OP.is_gt, op1=OP.mult)
                # scale + out-DMA in halves to shorten the scale->DMA tail;
                # out-DMAs issue from the idle Pool queue so their waits on
                # the DVE scale don't serialize the Sync queue
                for h in range(2):
                    hs = slice(h * (D // 2), (h + 1) * (D // 2))
                    V.tensor_scalar(out=y_sb[:, hs], in0=y_sb[:, hs],
                                    scalar1=alm[:], scalar2=10.0,
                                    op0=OP.mult, op1=OP.mult)
                    nc.gpsimd.dma_start(out=out_d[i * P:(i + 1) * P, hs],
                                        in_=y_sb[:, hs])
    nc.finalize()
    return nc


_NC = None


def _get_nc():
    global _NC
    if _NC is None:
        _NC = build_nc()
    return _NC


def _prep_inputs(x: np.ndarray, weights: np.ndarray):
    xf = np.ascontiguousarray(x, dtype=np.float32).reshape(ROWS_TOTAL, D)
    # w[p, kc*256+j] = W[kc//2, j, (kc%2)*128+p]
    wt = (weights.astype(np.float32).transpose(0, 2, 1)   # [r, k, j]
          .reshape(R, 2, P, BS).transpose(2, 0, 1, 3)     # [p, r, c, j]
          .reshape(P, 2 * R * BS)).astype(ml_dtypes.bfloat16)
    wt = np.ascontiguousarray(wt)

    qx = np.einsum('ij,ij->i', xf.astype(np.float64), xf.astype(np.float64))
    uc = np.maximum(np.sqrt(qx), 1e-5)
    k1 = (np.minimum(0.1 * uc, CA) / uc).astype(np.float32)

    in_maps = []
    for cidx in range(N_CORES):
        xc = xf[cidx * ROWS_CORE:(cidx + 1) * ROWS_CORE]
        # xt[p, ((i*32 + kc)*128) + r] = xc[i*128 + r, kc*128 + p]
        xt = (xc.reshape(NT, P, NKC, P).transpose(3, 0, 2, 1)
              .reshape(P, NT * D)).astype(ml_dtypes.bfloat16)
        k1c = np.ascontiguousarray(
            k1[cidx * ROWS_CORE:(cidx + 1) * ROWS_CORE].reshape(NT, P).T)
        in_maps.append({
            "xt": np.ascontiguousarray(xt),
            "w": wt,
            "k1": k1c,
        })
    return in_maps


def kernel(x: np.ndarray, weights: np.ndarray) -> np.ndarray:
    nc = _get_nc()
    in_maps = _prep_inputs(x, np.asarray(weights))
    res = run_bass_kernel_spmd(nc, in_maps, list(range(N_CORES)))
    out = np.concatenate([res.results[i]["out"] for i in range(N_CORES)], axis=0)
    return out.reshape(x.shape).astype(np.float32, copy=False)


if __name__ == "__main__":
    xs = np.random.randn(4, 2048, D).astype(np.float32)
    ws = (np.broadcast_to(np.eye(BS, dtype=np.float32), (R, BS, BS))
          + 0.02 * np.random.randn(R, BS, BS).astype(np.float32))
    o = kernel(xs, ws)
    print("kernel ran, out shape", o.shape, o.dtype)


# revision 23
# speedup vs baseline: 1.1022x; 1.1022x over previous
"""Trainium2 Bass kernel for nn_BlockDiagonalLinear_text (hyperbolic block-diag linear).

Math: every per-row operation in the reference is a scalar row-scaling of
  y = x @ blockdiag(W_1..W_16).T
and the scalar chain collapses via artanh(tanh(t)) = t:
  out = 10 * clamp(y_n * k1, 1e-6, CB) / y_n * y     (y_n = ||y||)
  k1  = min(0.1*uc, CA) / uc,  uc = max(||x||, 1e-5)
  CA  = artanh(f32(1 - 1e-5))   (expmap tanh always lands in the artanh clip)
  CB  = artanh(f32(0.1) * f32(9.99))  (the _project maxnorm cancels pf*ttx)
k1 depends only on x, so it is precomputed on the host (one fused scalar
per row) alongside the input layout transform.

Device layout (per core, data-parallel over rows: 8192 rows -> 8 x 1024):
  xt  [128, 8*32*128] bf16 -- x pre-transposed on host so each matmul's
      stationary operand xt[:, i*4096 + kc*128 :+128] = x[tile rows, k-chunk].T
      DMAs at full rate (8 KiB/partition/tile contiguous), no PE transposes,
      no PSUM->SBUF cast copies.
  w   [128, 8192] bf16 -- w[p, kc*256+j] = W[kc//2, j, (kc%2)*128+p]
  k1  [128, 8] f32 per-tile row scalars
Per 128-row tile: 32 bf16 matmuls (2 per 256-col block) into 8 PSUM banks
[128,512]; ACT Square+accum per bank -> qy partials; DVE copies bank->SBUF;
tiny DVE chain -> alm; DVE scaled in-place; DMA out f32.
Single ACT table (Square/Rsqrt/Copy) -> zero ACT_TABLE_LOADs.
"""
import sys
import numpy as np

for _p in ("/opt/trn_rl_repo", "/root/.axon_site/_ro/trn_rl_repo"):
    if _p not in sys.path:
        sys.path.append(_p)

import ml_dtypes
import concourse.bass as bass
import concourse.bacc as bacc
import concourse.mybir as mybir
from concourse import tile
from concourse.bass_utils import run_bass_kernel_spmd

R, BS = 16, 256           # 16 diagonal blocks of 256x256
D = R * BS                # 4096
P = 128                   # partitions
N_CORES = 8
ROWS_TOTAL = 4 * 2048     # 8192
ROWS_CORE = ROWS_TOTAL // N_CORES   # 1024
NT = ROWS_CORE // P       # 8 tiles of 128 rows per core
NKC = D // P              # 32 k-chunks of 128
NB = 8                    # PSUM banks per tile (512 cols each)
BANK = 512

f32 = mybir.dt.float32
bf16 = mybir.dt.bfloat16
AF = mybir.ActivationFunctionType
OP = mybir.AluOpType
AX = None  # set lazily (bass_rust import)

CA = 6.10235526389634     # artanh(f32(1 - 1e-5))
CB = 3.800207607813536    # artanh(f32(0.1) * f32((1-1e-3)/0.1))


def build_nc():
    import bass_rust
    nc = bacc.Bacc()
    xt_d = nc.declare_dram_parameter("xt", [P, NT * D], bf16, isOutput=False)
    w_d = nc.declare_dram_parameter("w", [P, 2 * R * BS], bf16, isOutput=False)
    k1_d = nc.declare_dram_parameter("k1", [P, NT], f32, isOutput=False)
    out_d = nc.declare_dram_parameter("out", [ROWS_CORE, D], f32, isOutput=True)

    with tile.TileContext(nc) as tc:
        with (
            tc.tile_pool(name="xtpool", bufs=1) as xtpool,
            tc.tile_pool(name="wpool", bufs=1) as wpool,
            tc.tile_pool(name="kpool", bufs=1) as kpool,
            tc.tile_pool(name="ypool", bufs=4) as ypool,
            tc.tile_pool(name="ostage", bufs=3) as ostage,
            tc.tile_pool(name="scrpool", bufs=1) as scrpool,
            tc.tile_pool(name="stats", bufs=4) as stats,
            tc.tile_pool(name="pst", bufs=4, space="PSUM") as pst,
        ):
            # input DMAs, ordered so tile-0 bank-0 can start ASAP
            wts = [wpool.tile([P, 8 * BS], bf16, tag=f"w{g}", name=f"w_{g}")
                   for g in range(4)]
            xts = [xtpool.tile([P, D], bf16, tag=f"xt{i}", name=f"xt_{i}")
                   for i in range(NT)]
            k1_sb = kpool.tile([P, NT], f32, name="k1_sb")

            nc.sync.dma_start(out=wts[0][:], in_=w_d[:, 0:8 * BS])
            for g in range(4):  # xt0 in quarters for early dependency release
                nc.sync.dma_start(out=xts[0][:, g * 1024:(g + 1) * 1024],
                                  in_=xt_d[:, g * 1024:(g + 1) * 1024])
            for g in range(1, 4):
                nc.sync.dma_start(out=wts[g][:],
                                  in_=w_d[:, g * 8 * BS:(g + 1) * 8 * BS])
            nc.sync.dma_start(out=k1_sb[:], in_=k1_d[:])
            for i in range(1, NT):
                nc.sync.dma_start(out=xts[i][:], in_=xt_d[:, i * D:(i + 1) * D])

            scr = scrpool.tile([P, D], bf16, name="scr")

            def st(shape, tag):
                return stats.tile(shape, f32, tag=tag, name=tag)

            V = nc.vector
            for i in range(NT):
                # y staged in bf16: halves copy/square/scale input cost
                y_sb = ypool.tile([P, D], bf16, tag="y", name=f"y_{i}")
                o_sb = ostage.tile([P, D], f32, tag="o", name=f"o_{i}")
                for bb in range(NB // 2):
                    # 2-bank PSUM tile: 4 blocks, 8 matmuls
                    py = pst.tile([P, 2 * BANK], f32, tag="py", name=f"py_{i}_{bb}")
                    for blk in range(4):
                        r = 4 * bb + blk
                        for c in range(2):
                            kc = 2 * r + c
                            nc.tensor.matmul(
                                py[:, blk * BS:(blk + 1) * BS],
                                xts[i][:, kc * P:(kc + 1) * P],
                                wts[kc // 8][:, (kc % 8) * BS:(kc % 8 + 1) * BS],
                                start=(c == 0), stop=(c == 1),
                            )
                    # y copy (2 banks, f32->bf16): split across ACT and DVE
                    dst = y_sb[:, bb * 2 * BANK:(bb + 1) * 2 * BANK]
                    if bb < 2:
                        nc.scalar.activation(dst, py[:], AF.Copy)
                    else:
                        V.tensor_copy(dst, py[:])

                # ---- collapsed per-row chain ----
                # qy = sum(y^2) in one ACT pass over bf16 y
                qy = st([P, 1], "qy")
                nc.scalar.activation(scr[:], y_sb[:], AF.Square,
                                     accum_out=qy[:])
                qyc = st([P, 1], "qyc")
                V.tensor_scalar_max(qyc[:], qy[:], 1e-38)
                y_n = st([P, 1], "y_n")
                nc.scalar.activation(y_n[:], qyc[:], AF.Sqrt)
                ry = st([P, 1], "ry")
                V.reciprocal(ry[:], y_n[:])
                w2 = st([P, 1], "w2")
                V.tensor_tensor(w2[:], y_n[:], k1_sb[:, i:i + 1], OP.mult)
                g_ = st([P, 1], "g_")
                V.tensor_scalar(out=g_[:], in0=w2[:], scalar1=1e-6, scalar2=CB,
                                op0=OP.max, op1=OP.min)
                a_ = st([P, 1], "a_")
                V.tensor_tensor(a_[:], g_[:], ry[:], OP.mult)
                alm = st([P, 1], "alm")
                V.scalar_tensor_tensor(out=alm[:], in0=qy[:], scalar=0.0,
                                       in1=a_[:], op0=OP.is_gt, op1=OP.mult)
                # scale + out-DMA in halves to shorten the scale->DMA tail;
                # out-DMAs issue from the idle Pool queue so their waits on
                # the DVE scale don't serialize the Sync queue
                for h in range(2):
                    hs = slice(h * (D // 2), (h + 1) * (D // 2))
                    V.tensor_scalar(out=o_sb[:, hs], in0=y_sb[:, hs],
                                    scalar1=alm[:], scalar2=10.0,
                                    op0=OP.mult, op1=OP.mult)
                    nc.gpsimd.dma_start(out=out_d[i * P:(i + 1) * P, hs],
                                        in_=o_sb[:, hs])
    nc.finalize()
    return nc


_NC = None


def _get_nc():
    global _NC
    if _NC is None:
        _NC = build_nc()
    return _NC


def _prep_inputs(x: np.ndarray, weights: np.ndarray):
    xf = np.ascontiguousarray(x, dtype=np.float32).reshape(ROWS_TOTAL, D)
    # w[p, kc*256+j] = W[kc//2, j, (kc%2)*128+p]
    wt = (weights.astype(np.float32).transpose(0, 2, 1)   # [r, k, j]
          .reshape(R, 2, P, BS).transpose(2, 0, 1, 3)     # [p, r, c, j]
          .reshape(P, 2 * R * BS)).astype(ml_dtypes.bfloat16)
    wt = np.ascontiguousarray(wt)

    qx = np.einsum('ij,ij->i', xf.astype(np.float64), xf.astype(np.float64))
    uc = np.maximum(np.sqrt(qx), 1e-5)
    k1 = (np.minimum(0.1 * uc, CA) / uc).astype(np.float32)

    in_maps = []
    for cidx in range(N_CORES):
        xc = xf[cidx * ROWS_CORE:(cidx + 1) * ROWS_CORE]
        # xt[p, ((i*32 + kc)*128) + r] = xc[i*128 + r, kc*128 + p]
        xt = (xc.reshape(NT, P, NKC, P).transpose(3, 0, 2, 1)
              .reshape(P, NT * D)).astype(ml_dtypes.bfloat16)
        k1c = np.ascontiguousarray(
            k1[cidx * ROWS_CORE:(cidx + 1) * ROWS_CORE].reshape(NT, P).T)
        in_maps.append({
            "xt": np.ascontiguousarray(xt),
            "w": wt,
            "k1": k1c,
        })
    return in_maps


def kernel(x: np.ndarray, weights: np.ndarray) -> np.ndarray:
    nc = _get_nc()
    in_maps = _prep_inputs(x, np.asarray(weights))
    res = run_bass_kernel_spmd(nc, in_maps, list(range(N_CORES)))
    out = np.concatenate([res.results[i]["out"] for i in range(N_CORES)], axis=0)
    return out.reshape(x.shape).astype(np.float32, copy=False)


if __name__ == "__main__":
    xs = np.random.randn(4, 2048, D).astype(np.float32)
    ws = (np.broadcast_to(np.eye(BS, dtype=np.float32), (R, BS, BS))
          + 0.02 * np.random.randn(R, BS, BS).astype(np.float32))
    o = kernel(xs, ws)
    print("kernel ran, out shape", o.shape, o.dtype)


# revision 26
# speedup vs baseline: 1.2963x; 1.1761x over previous
"""Trainium2 Bass kernel for nn_BlockDiagonalLinear_text (hyperbolic block-diag linear).

Math: every per-row operation in the reference is a scalar row-scaling of
  y = x @ blockdiag(W_1..W_16).T
and the scalar chain collapses via artanh(tanh(t)) = t:
  out = 10 * clamp(y_n * k1, 1e-6, CB) / y_n * y     (y_n = ||y||)
  k1  = min(0.1*uc, CA) / uc,  uc = max(||x||, 1e-5)
  CA  = artanh(f32(1 - 1e-5))   (expmap tanh always lands in the artanh clip)
  CB  = artanh(f32(0.1) * f32(9.99))  (the _project maxnorm cancels pf*ttx)
k1 depends only on x, so it is precomputed on the host (one fused scalar
per row) alongside the input layout transform.

Device layout (per core, data-parallel over rows: 8192 rows -> 8 x 1024):
  xt  [128, 8*32*128] bf16 -- x pre-transposed on host so each matmul's
      stationary operand xt[:, i*4096 + kc*128 :+128] = x[tile rows, k-chunk].T
      DMAs at full rate (8 KiB/partition/tile contiguous), no PE transposes,
      no PSUM->SBUF cast copies.
  w   [128, 8192] bf16 -- w[p, kc*256+j] = W[kc//2, j, (kc%2)*128+p]
  k1  [128, 8] f32 per-tile row scalars
Per 128-row tile: 32 bf16 matmuls (2 per 256-col block) into 8 PSUM banks
[128,512]; ACT Square+accum per bank -> qy partials; DVE copies bank->SBUF;
tiny DVE chain -> alm; DVE scaled in-place; DMA out f32.
Single ACT table (Square/Rsqrt/Copy) -> zero ACT_TABLE_LOADs.
"""
import sys
import numpy as np

for _p in ("/opt/trn_rl_repo", "/root/.axon_site/_ro/trn_rl_repo"):
    if _p not in sys.path:
        sys.path.append(_p)

import ml_dtypes
import concourse.bass as bass
import concourse.bacc as bacc
import concourse.mybir as mybir
from concourse import tile
from concourse.bass_utils import run_bass_kernel_spmd

R, BS = 16, 256           # 16 diagonal blocks of 256x256
D = R * BS                # 4096
P = 128                   # partitions
N_CORES = 8
ROWS_TOTAL = 4 * 2048     # 8192
ROWS_CORE = ROWS_TOTAL // N_CORES   # 1024
NT = ROWS_CORE // P       # 8 tiles of 128 rows per core
NKC = D // P              # 32 k-chunks of 128
NB = 8                    # PSUM banks per tile (512 cols each)
BANK = 512

f32 = mybir.dt.float32
bf16 = mybir.dt.bfloat16
AF = mybir.ActivationFunctionType
OP = mybir.AluOpType
AX = None  # set lazily (bass_rust import)

CA = 6.10235526389634     # artanh(f32(1 - 1e-5))
CB = 3.800207607813536    # artanh(f32(0.1) * f32((1-1e-3)/0.1))


def build_nc():
    import bass_rust
    nc = bacc.Bacc()
    xt_d = nc.declare_dram_parameter("xt", [P, NT * D], bf16, isOutput=False)
    w_d = nc.declare_dram_parameter("w", [P, 2 * R * BS], bf16, isOutput=False)
    k1_d = nc.declare_dram_parameter("k1", [P, NT], f32, isOutput=False)
    # bf16 device output (host upcasts): halves the output wire traffic
    out_d = nc.declare_dram_parameter("out", [ROWS_CORE, D], bf16, isOutput=True)

    with tile.TileContext(nc) as tc:
        with (
            tc.tile_pool(name="xtpool", bufs=1) as xtpool,
            tc.tile_pool(name="wpool", bufs=1) as wpool,
            tc.tile_pool(name="kpool", bufs=1) as kpool,
            tc.tile_pool(name="ypool", bufs=4) as ypool,
            tc.tile_pool(name="ostage", bufs=3) as ostage,
            tc.tile_pool(name="scrpool", bufs=1) as scrpool,
            tc.tile_pool(name="stats", bufs=4) as stats,
            tc.tile_pool(name="pst", bufs=4, space="PSUM") as pst,
        ):
            # input DMAs, ordered so tile-0 bank-0 can start ASAP
            wts = [wpool.tile([P, 8 * BS], bf16, tag=f"w{g}", name=f"w_{g}")
                   for g in range(4)]
            xts = [xtpool.tile([P, D], bf16, tag=f"xt{i}", name=f"xt_{i}")
                   for i in range(NT)]
            k1_sb = kpool.tile([P, NT], f32, name="k1_sb")

            nc.sync.dma_start(out=wts[0][:], in_=w_d[:, 0:8 * BS])
            for g in range(4):  # xt0 in quarters for early dependency release
                nc.sync.dma_start(out=xts[0][:, g * 1024:(g + 1) * 1024],
                                  in_=xt_d[:, g * 1024:(g + 1) * 1024])
            for g in range(1, 4):
                nc.sync.dma_start(out=wts[g][:],
                                  in_=w_d[:, g * 8 * BS:(g + 1) * 8 * BS])
            nc.sync.dma_start(out=k1_sb[:], in_=k1_d[:])
            for i in range(1, NT):
                nc.sync.dma_start(out=xts[i][:], in_=xt_d[:, i * D:(i + 1) * D])

            scr = scrpool.tile([P, D], bf16, name="scr")

            def st(shape, tag):
                return stats.tile(shape, f32, tag=tag, name=tag)

            V = nc.vector
            for i in range(NT):
                # y staged in bf16: halves copy/square/scale input cost
                y_sb = ypool.tile([P, D], bf16, tag="y", name=f"y_{i}")
                o_sb = ostage.tile([P, D], bf16, tag="o", name=f"o_{i}")
                for bb in range(NB // 2):
                    # 2-bank PSUM tile: 4 blocks, 8 matmuls
                    py = pst.tile([P, 2 * BANK], f32, tag="py", name=f"py_{i}_{bb}")
                    for blk in range(4):
                        r = 4 * bb + blk
                        for c in range(2):
                            kc = 2 * r + c
                            nc.tensor.matmul(
                                py[:, blk * BS:(blk + 1) * BS],
                                xts[i][:, kc * P:(kc + 1) * P],
                                wts[kc // 8][:, (kc % 8) * BS:(kc % 8 + 1) * BS],
                                start=(c == 0), stop=(c == 1),
                            )
                    # y copy (2 banks, f32->bf16): split across ACT and DVE
                    dst = y_sb[:, bb * 2 * BANK:(bb + 1) * 2 * BANK]
                    if bb < 2:
                        nc.scalar.activation(dst, py[:], AF.Copy)
                    else:
                        V.tensor_copy(dst, py[:])

                # ---- collapsed per-row chain ----
                # qy = sum(y^2) in one ACT pass over bf16 y
                qy = st([P, 1], "qy")
                nc.scalar.activation(scr[:], y_sb[:], AF.Square,
                                     accum_out=qy[:])
                qyc = st([P, 1], "qyc")
                V.tensor_scalar_max(qyc[:], qy[:], 1e-38)
                y_n = st([P, 1], "y_n")
                nc.scalar.activation(y_n[:], qyc[:], AF.Sqrt)
                ry = st([P, 1], "ry")
                V.reciprocal(ry[:], y_n[:])
                w2 = st([P, 1], "w2")
                V.tensor_tensor(w2[:], y_n[:], k1_sb[:, i:i + 1], OP.mult)
                g_ = st([P, 1], "g_")
                V.tensor_scalar(out=g_[:], in0=w2[:], scalar1=1e-6, scalar2=CB,
                                op0=OP.max, op1=OP.min)
                a_ = st([P, 1], "a_")
                V.tensor_tensor(a_[:], g_[:], ry[:], OP.mult)
                alm = st([P, 1], "alm")
                V.scalar_tensor_tensor(out=alm[:], in0=qy[:], scalar=0.0,
                                       in1=a_[:], op0=OP.is_gt, op1=OP.mult)
                # scale + out-DMA in halves to shorten the scale->DMA tail;
                # out-DMAs issue from the idle Pool queue so their waits on
                # the DVE scale don't serialize the Sync queue
                for h in range(2):
                    hs = slice(h * (D // 2), (h + 1) * (D // 2))
                    V.tensor_scalar(out=o_sb[:, hs], in0=y_sb[:, hs],
                                    scalar1=alm[:], scalar2=10.0,
                                    op0=OP.mult, op1=OP.mult)
                    nc.gpsimd.dma_start(out=out_d[i * P:(i + 1) * P, hs],
                                        in_=o_sb[:, hs])
    nc.finalize()
    return nc


_NC = None


def _get_nc():
    global _NC
    if _NC is None:
        _NC = build_nc()
    return _NC


def _prep_inputs(x: np.ndarray, weights: np.ndarray):
    xf = np.ascontiguousarray(x, dtype=np.float32).reshape(ROWS_TOTAL, D)
    # w[p, kc*256+j] = W[kc//2, j, (kc%2)*128+p]
    wt = (weights.astype(np.float32).transpose(0, 2, 1)   # [r, k, j]
          .reshape(R, 2, P, BS).transpose(2, 0, 1, 3)     # [p, r, c, j]
          .reshape(P, 2 * R * BS)).astype(ml_dtypes.bfloat16)
    wt = np.ascontiguousarray(wt)

    qx = np.einsum('ij,ij->i', xf.astype(np.float64), xf.astype(np.float64))
    uc = np.maximum(np.sqrt(qx), 1e-5)
    k1 = (np.minimum(0.1 * uc, CA) / uc).astype(np.float32)

    in_maps = []
    for cidx in range(N_CORES):
        xc = xf[cidx * ROWS_CORE:(cidx + 1) * ROWS_CORE]
        # xt[p, ((i*32 + kc)*128) + r] = xc[i*128 + r, kc*128 + p]
        xt = (xc.reshape(NT, P, NKC, P).transpose(3, 0, 2, 1)
              .reshape(P, NT * D)).astype(ml_dtypes.bfloat16)
        k1c = np.ascontiguousarray(
            k1[cidx * ROWS_CORE:(cidx + 1) * ROWS_CORE].reshape(NT, P).T)
        in_maps.append({
            "xt": np.ascontiguousarray(xt),
            "w": wt,
            "k1": k1c,
        })
    return in_maps


def kernel(x: np.ndarray, weights: np.ndarray) -> np.ndarray:
    nc = _get_nc()
    in_maps = _prep_inputs(x, np.asarray(weights))
    res = run_bass_kernel_spmd(nc, in_maps, list(range(N_CORES)))
    out = np.concatenate([np.asarray(res.results[i]["out"])
                          for i in range(N_CORES)], axis=0)
    return out.reshape(x.shape).astype(np.float32)


if __name__ == "__main__":
    xs = np.random.randn(4, 2048, D).astype(np.float32)
    ws = (np.broadcast_to(np.eye(BS, dtype=np.float32), (R, BS, BS))
          + 0.02 * np.random.randn(R, BS, BS).astype(np.float32))
    o = kernel(xs, ws)
    print("kernel ran, out shape", o.shape, o.dtype)
